# revision 11
# baseline (speedup 1.0000x reference)
"""DeformConv2D Trainium2 kernel (8-core batch-parallel).

Algorithm per core (one batch image):
  1. Host supplies x as a zero-margin-extended spatial-major image
     xe [SE, C] in DRAM, offsets pre-laid-out [128, 2*NF], a constant
     base grid, and pre-transposed conv weights.
  2. Device computes bilinear corner indices + weights from offsets
     (exact reference semantics incl. the pad-snap quirk; out-of-range
     corners hit zero margin so clipping is unnecessary).
  3. SWDGE dma_gather pulls 2x512-float rows per sample (two corner
     pairs), blend on DVE with per-partition scalars, PE-transpose to
     [c, pix], then fp32r matmul accumulates the 3x3 deformable conv.
"""
import sys
import numpy as np

sys.path.insert(0, "/opt/trn_rl_repo")

KS, PAD = 3, 1
B, C, H, W = 8, 256, 64, 64
OUTC = 256
N = KS * KS              # 9 taps
HP = H + 2 * PAD         # 66  (xp frame rows 0..65)
MARG = 8                 # extra zero margin beyond xp frame
HE = HP + 2 * MARG       # 82
SE = HE * HE             # 6724 spatial rows in xe
NPIX = H * W             # 4096
NS = N * NPIX            # 36864 samples per batch
NF = NS // 128           # 288 free cols; sample s = f*128 + p, f = pixblk*9 + t
NCHUNK = NPIX // 128     # 32 chunks (1 pixel-block each)
IDX_PER_CHUNK = 2 * N * 128          # 2304 gather indices per chunk
TBL_COLS = IDX_PER_CHUNK // 16       # 144 idx-table cols per chunk

_BUILT = None


def _build(num_devices=8, nchunks=NCHUNK, mode="full", gsplit=3, single_packet=True):
    import concourse.bass as bass
    import concourse.bacc as bacc
    import concourse.mybir as mybir
    import concourse.tile as tile
    import concourse.masks as masks

    dt = mybir.dt
    alu = mybir.AluOpType

    nc = bacc.Bacc("TRN2", target_bir_lowering=False, debug=False,
                   num_devices=num_devices)

    i_xe = nc.dram_tensor("xe", [SE, C], dt.float32, kind="ExternalInput").ap()
    i_off = nc.dram_tensor("off", [128, 2 * NF], dt.float32, kind="ExternalInput").ap()
    i_grid = nc.dram_tensor("grid", [128, 2 * NF], dt.float32, kind="ExternalInput").ap()
    i_wt = nc.dram_tensor("wt", [2 * N, 128, OUTC], dt.float32, kind="ExternalInput").ap()
    o_out = nc.dram_tensor("out", [2, 128, NPIX], dt.float32, kind="ExternalOutput").ap()
    i_tbl = None
    if mode == "hosttbl":
        i_tbl = nc.dram_tensor("tbl", [128, NCHUNK * TBL_COLS], dt.int16,
                               kind="ExternalInput").ap()

    xe_view = bass.AP(i_xe.tensor, 0, [[C, SE - 1], [1, 2 * C]])

    with tile.TileContext(nc) as tc:
        with (
            tc.tile_pool(name="const", bufs=1) as cp,
            tc.tile_pool(name="scr", bufs=1) as sp,
            tc.tile_pool(name="gat", bufs=2) as gp,
            tc.tile_pool(name="xo", bufs=3) as xp_,
            tc.tile_pool(name="xoT", bufs=2) as tp,
            tc.tile_pool(name="osb", bufs=2) as op_,
            tc.tile_pool(name="pst", bufs=2, space="PSUM") as pst,
            tc.tile_pool(name="psm", bufs=2, space="PSUM") as psm,
        ):
            # ---------- constants / inputs ----------
            wt = cp.tile([128, 2 * N, OUTC], dt.float32r)
            with tc.tile_pool(name="tmpw", bufs=1) as twp:
                wt_f = twp.tile([128, 2 * N, OUTC], dt.float32)
                nc.sync.dma_start(wt_f[:], i_wt.transpose([1, 0, 2]))
                nc.vector.tensor_copy(wt[:], wt_f[:])

            ident_f = cp.tile([128, 128], dt.float32)
            masks.make_identity(nc, ident_f[:])
            ident = cp.tile([128, 128], dt.float32r)
            nc.vector.tensor_copy(ident[:], ident_f[:])

            off = cp.tile([128, 2 * NF], dt.float32)
            grid = cp.tile([128, 2 * NF], dt.float32)
            nc.sync.dma_start(off[:], i_off)
            nc.sync.dma_start(grid[:], i_grid)

            # ---------- offset math (both coords at once on [128, 576]) ----
            p = sp.tile([128, 2 * NF], dt.float32, name="p", tag="sA")
            nc.vector.tensor_tensor(p[:], grid[:], off[:], alu.add)
            ri = sp.tile([128, 2 * NF], dt.int32, name="ri", tag="sB")
            nc.vector.tensor_copy(ri[:], p[:])
            rf = sp.tile([128, 2 * NF], dt.float32, name="rf", tag="sC")
            nc.vector.tensor_copy(rf[:], ri[:])
            gtv = sp.tile([128, 2 * NF], dt.float32, name="gtv", tag="sD")
            nc.vector.tensor_tensor(gtv[:], rf[:], p[:], alu.is_gt)
            fl = sp.tile([128, 2 * NF], dt.float32, name="fl", tag="sE")
            nc.vector.tensor_tensor(fl[:], rf[:], gtv[:], alu.subtract)
            # in-bounds mask (pad-snap): inb = (p >= 1) * (p <= 64)
            t1 = sp.tile([128, 2 * NF], dt.float32, name="t1", tag="sB")
            nc.vector.tensor_scalar(t1[:], p[:], 1.0, None, alu.is_ge)
            t2 = sp.tile([128, 2 * NF], dt.float32, name="t2", tag="sC")
            nc.vector.tensor_scalar(t2[:], p[:], float(HP - 2), None, alu.is_le)
            nc.vector.tensor_tensor(t1[:], t1[:], t2[:], alu.mult)
            # snapped fractional part fs = inb * (p - fl); w1 == fs
            fr = sp.tile([128, 2 * NF], dt.float32, name="fr", tag="sD")
            nc.vector.tensor_tensor(fr[:], p[:], fl[:], alu.subtract)
            w1 = sp.tile([128, 2 * NF], dt.float32, name="w1", tag="sA")
            nc.vector.tensor_tensor(w1[:], fr[:], t1[:], alu.mult)
            w0 = sp.tile([128, 2 * NF], dt.float32, name="w0", tag="sC")
            nc.vector.tensor_scalar(w0[:], w1[:], -1.0, 1.0, alu.mult, alu.add)
            # corner products
            w00 = cp.tile([128, NF], dt.float32)
            w01 = cp.tile([128, NF], dt.float32)
            w10 = cp.tile([128, NF], dt.float32)
            w11 = cp.tile([128, NF], dt.float32)
            nc.vector.tensor_tensor(w00[:], w0[:, :NF], w0[:, NF:], alu.mult)
            nc.vector.tensor_tensor(w01[:], w0[:, :NF], w1[:, NF:], alu.mult)
            nc.vector.tensor_tensor(w10[:], w1[:, :NF], w0[:, NF:], alu.mult)
            nc.vector.tensor_tensor(w11[:], w1[:, :NF], w1[:, NF:], alu.mult)
            # clamped integer corner base: ixc, iyc in [-MARG, HP-2+MARG]
            ic = sp.tile([128, 2 * NF], dt.float32, name="ic", tag="sB")
            nc.vector.tensor_scalar(ic[:], fl[:], float(HP - 2 + MARG),
                                    float(-MARG), alu.min, alu.max)
            # flat row index idx0 = (ixc+MARG)*HE + iyc+MARG
            idxf = sp.tile([128, NF], dt.float32, name="idxf", tag="sF")
            nc.vector.tensor_scalar(idxf[:], ic[:, :NF], float(HE),
                                    float(MARG * HE + MARG), alu.mult, alu.add)
            nc.vector.tensor_tensor(idxf[:], idxf[:], ic[:, NF:], alu.add)

            # ---------- fold to 16-part idx table ----------
            fold = sp.tile([16, NF * 8], dt.float32)
            for q in range(8):
                nc.sync.dma_start(
                    fold[:, q:NF * 8:8].unsqueeze(-1),
                    idxf[q * 16:(q + 1) * 16, :].unsqueeze(-1),
                )
            table = cp.tile([128, NCHUNK * TBL_COLS], dt.int16)
            t0_ap = bass.AP(table.tensor, table[:16].offset,
                            [table[:16].ap[0], [TBL_COLS, NCHUNK], [1, 72]])
            f_ap = bass.AP(fold.tensor, fold[:].offset,
                           [fold[:].ap[0], [72, NCHUNK], [1, 72]])
            nc.vector.tensor_copy(t0_ap, f_ap)
            t1_ap = bass.AP(table.tensor, table[:16].offset + 72,
                            [table[:16].ap[0], [TBL_COLS, NCHUNK], [1, 72]])
            nc.vector.tensor_scalar(t1_ap, f_ap, float(HE), None, alu.add)
            # replicate table rows 0-15 across all 8 16-partition groups
            for rep in range(3):
                span = 16 << rep
                nc.sync.dma_start(table[span:2 * span, :], table[:span, :])

            if mode == "hosttbl":
                htbl = cp.tile([128, NCHUNK * TBL_COLS], dt.int16)
                nc.sync.dma_start(htbl[:], i_tbl)

            # ---------- main pipeline ----------
            for sc in range(nchunks // 2):
                xoT = {cb: tp.tile([128, N, 256], dt.float32r,
                                   name=f"xoT{cb}", tag=f"xoT{cb}")
                       for cb in range(2)}
                for half in range(2):
                    ch = 2 * sc + half
                    g = gp.tile([128, 2 * N, 2 * C], dt.float32)
                    if mode == "nogather":
                        nc.gpsimd.memset(g[:], 0.25)
                    else:
                        src_tbl = htbl if mode == "hosttbl" else table
                        nidx = IDX_PER_CHUNK // gsplit
                        tcols = TBL_COLS // gsplit
                        ngrp = nidx // 128
                        for gs in range(gsplit):
                            nc.gpsimd.dma_gather(
                                g[:, gs * ngrp:(gs + 1) * ngrp, :], xe_view,
                                src_tbl[:, ch * TBL_COLS + gs * tcols:
                                        ch * TBL_COLS + (gs + 1) * tcols],
                                num_idxs=nidx, num_idxs_reg=nidx,
                                elem_size=2 * C, elem_step=C,
                                single_packet=single_packet,
                            )
                    for t in range(N):
                        wc = ch * N + t
                        xo = xp_.tile([128, 256], dt.float32, tag="xo")
                        nc.vector.tensor_scalar(
                            xo[:], g[:, t, 0:256], w00[:, wc:wc + 1], None,
                            alu.mult)
                        nc.vector.scalar_tensor_tensor(
                            xo[:], g[:, t, 256:512], w01[:, wc:wc + 1], xo[:],
                            alu.mult, alu.add)
                        nc.vector.scalar_tensor_tensor(
                            xo[:], g[:, N + t, 0:256], w10[:, wc:wc + 1], xo[:],
                            alu.mult, alu.add)
                        xor_ = xp_.tile([128, 256], dt.float32r, tag="xor")
                        nc.vector.scalar_tensor_tensor(
                            xor_[:], g[:, N + t, 256:512], w11[:, wc:wc + 1],
                            xo[:], alu.mult, alu.add)
                        for cb in range(2):
                            ptr = pst.tile([128, 128], dt.float32r, tag="ptr")
                            nc.tensor.transpose(
                                ptr[:], xor_[:, cb * 128:(cb + 1) * 128],
                                ident[:])
                            nc.scalar.copy(
                                xoT[cb][:, t, half * 128:(half + 1) * 128],
                                ptr[:])
                for hf in range(2):
                    pm = psm.tile([128, 256], dt.float32, tag="pm")
                    for kt in range(2 * N):
                        t, cb = kt // 2, kt % 2
                        nc.tensor.matmul(
                            pm[:], wt[:, kt, hf * 128:(hf + 1) * 128],
                            xoT[cb][:, t, :],
                            start=(kt == 0), stop=(kt == 2 * N - 1))
                    ob = op_.tile([128, 256], dt.float32, tag="ob")
                    nc.scalar.copy(ob[:], pm[:])
                    nc.sync.dma_start(o_out[hf, :, sc * 256:(sc + 1) * 256], ob[:])

    nc.compile()
    return nc


def _host_prep(x, offset, weight):
    xe = np.zeros((B, HE, HE, C), dtype=np.float32)
    xe[:, MARG + 1:MARG + 1 + H, MARG + 1:MARG + 1 + W, :] = \
        x.transpose(0, 2, 3, 1)
    xe = xe.reshape(B, SE, C)

    def lay(o):  # [B, N, H, W] -> [B, 128, NF] with s=(pixblk*9+t)*128+p
        o = o.reshape(B, N, NPIX // 128, 128)
        return np.ascontiguousarray(
            o.transpose(0, 2, 1, 3).reshape(B, NF, 128).transpose(0, 2, 1))

    ox = lay(offset[:, 0::2])
    oy = lay(offset[:, 1::2])
    off = np.concatenate([ox, oy], axis=2)          # [B, 128, 2*NF]

    r = np.arange(-(KS - 1) // 2, (KS - 1) // 2 + 1)
    pnx, pny = np.meshgrid(r, r, indexing="ij")
    i_idx, j_idx = np.meshgrid(np.arange(1, H + 1), np.arange(1, W + 1),
                               indexing="ij")
    gx = (i_idx.reshape(-1).astype(np.float32).reshape(NPIX // 128, 1, 128)
          + pnx.reshape(-1).astype(np.float32).reshape(1, N, 1))
    gy = (j_idx.reshape(-1).astype(np.float32).reshape(NPIX // 128, 1, 128)
          + pny.reshape(-1).astype(np.float32).reshape(1, N, 1))
    gx = gx.reshape(NF, 128).T
    gy = gy.reshape(NF, 128).T
    grid = np.ascontiguousarray(np.concatenate([gx, gy], axis=1))  # [128,2NF]

    wt = weight.reshape(OUTC, C, N).transpose(2, 1, 0)      # [t, c, o]
    wt = np.ascontiguousarray(
        wt.reshape(N, 2, 128, OUTC).reshape(2 * N, 128, OUTC))
    return xe, off, grid, wt


def kernel(x, offset, weight):
    global _BUILT
    from concourse.bass_utils import run_bass_kernel_spmd

    x = np.asarray(x, dtype=np.float32)
    offset = np.asarray(offset, dtype=np.float32)
    weight = np.asarray(weight, dtype=np.float32)

    xe, off, grid, wt = _host_prep(x, offset, weight)
    if _BUILT is None:
        _BUILT = _build()
    nc = _BUILT

    in_maps = [
        {"xe": xe[b], "off": off[b], "grid": grid, "wt": wt}
        for b in range(B)
    ]
    res = run_bass_kernel_spmd(nc, in_maps, list(range(B)))
    out = np.stack([
        res.results[b]["out"].reshape(OUTC, H, W) for b in range(B)
    ])
    return out
